# revision 8
# baseline (speedup 1.0000x reference)
"""Trainium2 Bass kernel for nn_BeatAlignmentModule (cross-attention alignment
loss). Self-contained: takes full inputs, shards over 8 NeuronCores, returns
(loss, att_maps) matching the reference.

Math per (i, j) pair (i = sentence sample, j = ecg sample, all L=64, D=512):
  S[s,q]   = <ecg_j[s,:], sent_i[q,:]>                  (PE, bf16 split-3)
  attn0    = softmax_q(S)       = E0 * rinv0            (ACT exp + DVE)
  X        = exp(T1 * attn0)                            (ACT)
  N[q]     = sum_s X[s,q] * S[s,q]       (numerator <sent, wctx>)
  u[q]     = sum_s X[s,q] * (G_j X)[s,q] (||wctx||^2 via Gram trick)
  cos[q]   = N / (w1 * sqrt(u))   (softmax2 normalizer cancels in cos)
  z[j,i]   = sum_q exp(T2 * cos)
Host: sim = T3*ln(z), symmetric CE loss; att_maps from exported E0 diagonals.

Sharding: data-parallel over i (8 per core); each core sees all j but with a
per-core permutation so its own diagonal j's occupy slots 0..7 (keeps the
device program SPMD-uniform for att-map extraction).
"""
import os
import sys

sys.path.insert(0, "/opt/trn_rl_repo")
os.environ.setdefault("JAX_PLATFORMS", "axon,cpu")

import numpy as np
import ml_dtypes
from contextlib import ExitStack

import concourse.bass as bass
import concourse.tile as tile
from concourse import bacc, mybir
from concourse.bass_utils import run_bass_kernel_spmd

B, L, D = 64, 64, 512
N_CORES = 8
IPC = B // N_CORES            # i's per core
NJP = B // 2                  # j-pairs per core
TEMP1, TEMP2, TEMP3 = 4.0, 5.0, 10.0
CSHIFT = 70.0                 # constant shift inside exp(S - CSHIFT)

F32 = mybir.dt.float32
F32R = mybir.dt.float32r
BF16 = mybir.dt.bfloat16
EXP = mybir.ActivationFunctionType.Exp
LOG = mybir.ActivationFunctionType.Ln
MULT = mybir.AluOpType.mult
ADD = mybir.AluOpType.add
AXX = mybir.AxisListType.X

_CACHE = {}


def _build_program():
    nc = bacc.Bacc("TRN2", target_bir_lowering=False, debug=False,
                   enable_asserts=True, num_devices=N_CORES)

    cb = nc.alloc_sbuf_tensor("const-mshift", [128, 1], F32)
    nc.gpsimd.memset(cb.ap(), -CSHIFT)
    nc.const_aps.aps[(F32, -CSHIFT)] = cb.ap()
    nc.all_engine_barrier()

    # ---- DRAM I/O ----
    ehi_d = nc.dram_tensor("ehi", [D, B * L], BF16, kind="ExternalInput").ap()
    elo_d = nc.dram_tensor("elo", [D, B * L], BF16, kind="ExternalInput").ap()
    shi_d = nc.dram_tensor("shi", [D, IPC * L], BF16, kind="ExternalInput").ap()
    slo_d = nc.dram_tensor("slo", [D, IPC * L], BF16, kind="ExternalInput").ap()
    w1i_d = nc.dram_tensor("w1i", [64, IPC * L], F32, kind="ExternalInput").ap()
    dlt_d = nc.dram_tensor("dlt", [IPC, IPC * L], F32R, kind="ExternalInput").ap()
    brt_d = nc.dram_tensor("brt", [128, 254], F32R, kind="ExternalInput").ap()
    idn_d = nc.dram_tensor("idn", [128, 128], F32, kind="ExternalInput").ap()
    zro_d = nc.dram_tensor("zro", [128, 128], F32R, kind="ExternalInput").ap()

    z_d = nc.dram_tensor("zout", [64, IPC], F32, kind="ExternalOutput").ap()
    att_d = nc.dram_tensor("attout", [IPC, L, L], F32, kind="ExternalOutput").ap()
    dbg = {}
    if os.environ.get("BAK_DEBUG", "0") == "1":
        for nm, shp in [("d_at", [128, 512]), ("d_x", [128, 512]),
                        ("d_p1", [128, 512]), ("d_p2", [128, 512]),
                        ("d_num", [64, 512]), ("d_uu", [64, 512]),
                        ("d_cos", [64, 512]), ("d_rt", [IPC, 128]),
                        ("d_gsb", [128, 128])]:
            dbg[nm] = nc.dram_tensor(nm, shp, F32, kind="ExternalOutput").ap()

    NC, NF = 128, IPC * L  # partition count, wide free dim (512)

    with tile.TileContext(nc) as tc, ExitStack() as ctx:
        cst = ctx.enter_context(tc.tile_pool(name="cst", bufs=1))
        wrk = ctx.enter_context(tc.tile_pool(name="wrk", bufs=2))
        wrk3 = ctx.enter_context(tc.tile_pool(name="wrk3", bufs=3))
        psS = ctx.enter_context(tc.tile_pool(name="psS", bufs=2, space="PSUM"))
        psY = ctx.enter_context(tc.tile_pool(name="psY", bufs=2, space="PSUM"))
        psR = ctx.enter_context(tc.tile_pool(name="psR", bufs=1, space="PSUM"))
        psG = ctx.enter_context(tc.tile_pool(name="psG", bufs=1, space="PSUM"))
        psN = ctx.enter_context(tc.tile_pool(name="psN", bufs=1, space="PSUM"))
        psU = ctx.enter_context(tc.tile_pool(name="psU", bufs=1, space="PSUM"))

        # ---- constants / inputs to SBUF ----
        ident = cst.tile([128, 128], F32, tag="ident")
        nc.sync.dma_start(ident[:], idn_d[:])
        delta = cst.tile([IPC, NF], F32R, tag="delta")
        nc.sync.dma_start(delta[:], dlt_d[:])
        broute = cst.tile([128, 254], F32R, tag="broute")
        nc.sync.dma_start(broute[:], brt_d[:])
        w1inv = cst.tile([64, NF], F32, tag="w1inv")
        nc.sync.dma_start(w1inv[:], w1i_d[:])

        gsb = [cst.tile([128, 128], F32R, tag=f"gsb{k}", name=f"gsb{k}") for k in range(2)]
        nc.sync.dma_start(gsb[0][:], zro_d[:])
        nc.sync.dma_start(gsb[1][:], zro_d[:])

        shi = []
        slo = []
        for c in range(4):
            th = cst.tile([128, NF], BF16, tag=f"shi{c}")
            nc.sync.dma_start(th[:], shi_d[128 * c:128 * c + 128, :])
            tl = cst.tile([128, NF], BF16, tag=f"slo{c}")
            nc.sync.dma_start(tl[:], slo_d[128 * c:128 * c + 128, :])
            shi.append(th)
            slo.append(tl)

        ehi = [cst.tile([128, B * L], BF16, tag=f"ehi{c}", name=f"ehi{c}") for c in range(4)]
        elo = [cst.tile([128, B * L], BF16, tag=f"elo{c}", name=f"elo{c}") for c in range(4)]
        NSL = 4  # dma column slices
        slw = (B * L) // NSL
        for s in range(NSL):
            for c in range(4):
                nc.sync.dma_start(ehi[c][:, slw * s:slw * (s + 1)],
                                  ehi_d[128 * c:128 * c + 128, slw * s:slw * (s + 1)])
                nc.sync.dma_start(elo[c][:, slw * s:slw * (s + 1)],
                                  elo_d[128 * c:128 * c + 128, slw * s:slw * (s + 1)])

        # ---- main loop over j-pairs ----
        pend = None  # deferred (x, p1, p2s, jj) for Y/colsum of previous jj
        nnum = psN.tile([128, NF], F32, tag="nnum")
        nuu = psU.tile([128, NF], F32, tag="nuu")

        def emit_deferred(p):
            x, p1t, jjp = p
            # Y = blockdiag(G) @ X
            py = psY.tile([128, NF], F32, tag="psy")
            nc.tensor.matmul(py[:], gsb[jjp % 2][:], x[:])
            # P2 = X * Y
            p2t = wrk.tile([128, NF], F32R, tag="p2")
            nc.vector.tensor_tensor(p2t[:], x[:], py[:], op=MULT)
            if dbg and jjp == 0:
                nc.sync.dma_start(dbg["d_p2"][:], p2t[:].bitcast(F32))
                nc.sync.dma_start(dbg["d_gsb"][:], gsb[jjp % 2][:].bitcast(F32))
            # colsum accumulations
            o = 126 - 2 * jjp
            nc.tensor.matmul(nnum[:], broute[:, o:o + 128], p1t[:],
                             start=(jjp == 0), stop=(jjp == NJP - 1),
                             skip_group_check=True)
            nc.tensor.matmul(nuu[:], broute[:, o:o + 128], p2t[:],
                             start=(jjp == 0), stop=(jjp == NJP - 1),
                             skip_group_check=True)

        for jj in range(NJP):
            jc = slice(128 * jj, 128 * jj + 128)
            # --- Gram of the ecg j-pair (bf16 hi only) ---
            pg = psG.tile([128, 128], F32, tag="pg")
            for c in range(4):
                nc.tensor.matmul(pg[:], ehi[c][:, jc], ehi[c][:, jc],
                                 start=(c == 0), stop=(c == 3))
            # copy diagonal blocks into the (pre-zeroed) block-diag tile
            nc.scalar.copy(gsb[jj % 2][0:64, 0:64], pg[0:64, 0:64])
            nc.scalar.copy(gsb[jj % 2][64:128, 64:128], pg[64:128, 64:128])

            # --- MM1: S = ecgT^T @ sentT, bf16 split-3 ---
            ps = psS.tile([128, NF], F32, tag="ps")
            k = 0
            for lh, rh in ((ehi, shi), (ehi, slo), (elo, shi)):
                for c in range(4):
                    nc.tensor.matmul(ps[:], lh[c][:, jc], rh[c][:],
                                     start=(k == 0), stop=(k == 11))
                    k += 1

            # --- deferred Y / colsums of previous jj (keeps PE busy while
            #     this jj's elementwise chain runs) ---
            if pend is not None:
                emit_deferred(pend)

            # --- softmax1 pieces ---
            e0r = wrk.tile([128, NF], F32, tag="e0r")
            nc.scalar.activation(e0r[:], ps[:], EXP, bias=-CSHIFT, scale=1.0)
            # clamp: exp can overflow to inf for |S| ~ 180; softmax saturates
            # so clamping is harmless (err ~1e-9). Runs on idle GPSIMD.
            e0 = wrk.tile([128, NF], F32, tag="e0")
            nc.gpsimd.tensor_scalar_min(e0[:], e0r[:], 1e35)
            r0 = wrk.tile([128, IPC], F32, tag="r0")
            nc.vector.tensor_reduce(r0[:], e0[:].rearrange("p (a b) -> p a b", a=IPC),
                                    axis=AXX, op=ADD)
            rinv = wrk.tile([128, IPC], F32, tag="rinv")
            nc.vector.reciprocal(rinv[:], r0[:])

            # att-map source export (diagonal pairs live in slots 0..7)
            if jj < IPC // 2:
                for h in (0, 1):
                    kb = 2 * jj + h
                    nc.sync.dma_start(att_d[kb],
                                      e0[64 * h:64 * h + 64, 64 * kb:64 * kb + 64])

            # --- broadcast rinv along q: PE transpose + selector matmul ---
            pr = psR.tile([128, NF], F32, tag="pr")
            nc.tensor.transpose(pr[0:IPC, 0:128], rinv[:], ident[:])
            rt = wrk.tile([IPC, 128], F32R, tag="rt")
            nc.scalar.copy(rt[:], pr[0:IPC, 0:128])
            nc.tensor.matmul(pr[:], rt[:], delta[:])

            at = wrk.tile([128, NF], F32, tag="at")
            nc.vector.tensor_tensor(at[:], e0[:], pr[:], op=MULT)
            if dbg and jj == 0:
                nc.sync.dma_start(dbg["d_at"][:], at[:])
                nc.sync.dma_start(dbg["d_rt"][:], rt[:].bitcast(F32))

            # --- X = exp(T1 * attn0) ---
            x = wrk3.tile([128, NF], F32R, tag="x")
            nc.scalar.activation(x[:], at[:], EXP, bias=0.0, scale=TEMP1)

            # --- P1 = X * S ---
            p1t = wrk3.tile([128, NF], F32R, tag="p1")
            nc.vector.tensor_tensor(p1t[:], x[:], ps[:], op=MULT)
            if dbg and jj == 0:
                nc.sync.dma_start(dbg["d_x"][:], x[:].bitcast(F32))
                nc.sync.dma_start(dbg["d_p1"][:], p1t[:].bitcast(F32))

            pend = (x, p1t, jj)

        emit_deferred(pend)

        # ---- tail: cos, z ----
        lu = wrk.tile([64, NF], F32, tag="lu")
        nc.scalar.activation(lu[:], nuu[0:64, :], LOG)
        isq = wrk.tile([64, NF], F32, tag="isq")
        nc.scalar.activation(isq[:], lu[:], EXP, bias=0.0, scale=-0.5)
        c1 = wrk.tile([64, NF], F32, tag="c1")
        nc.vector.tensor_tensor(c1[:], nnum[0:64, :], w1inv[:], op=MULT)
        cosv = wrk.tile([64, NF], F32, tag="cosv")
        nc.vector.tensor_tensor(cosv[:], c1[:], isq[:], op=MULT)
        if dbg:
            dnum = wrk.tile([64, NF], F32, tag="dnum")
            nc.vector.tensor_copy(dnum[:], nnum[0:64, :])
            nc.sync.dma_start(dbg["d_num"][:], dnum[:])
            duu = wrk.tile([64, NF], F32, tag="duu")
            nc.vector.tensor_copy(duu[:], nuu[0:64, :])
            nc.sync.dma_start(dbg["d_uu"][:], duu[:])
            nc.sync.dma_start(dbg["d_cos"][:], cosv[:])
        ez = wrk.tile([64, NF], F32, tag="ez")
        nc.scalar.activation(ez[:], cosv[:], EXP, bias=0.0, scale=TEMP2)
        zt = wrk.tile([64, IPC], F32, tag="zt")
        nc.vector.tensor_reduce(zt[:], ez[:].rearrange("p (a b) -> p a b", a=IPC),
                                axis=AXX, op=ADD)
        nc.sync.dma_start(z_d[:], zt[:])

    nc.compile()
    return nc


def _host_inputs(ecg, sent):
    """Build per-core input maps. ecg/sent: (B, L, D) float32."""
    bf16 = ml_dtypes.bfloat16
    ecgT = np.ascontiguousarray(ecg.transpose(2, 0, 1))          # (D, B, L)
    ehi = ecgT.astype(bf16)
    elo = (ecgT - ehi.astype(np.float32)).astype(bf16)

    brt = np.zeros((128, 254), dtype=np.float32)
    brt[0:64, 126] = 1.0
    brt[64:128, 127] = 1.0
    dlt = np.zeros((IPC, IPC * L), dtype=np.float32)
    for i in range(IPC):
        dlt[i, i * L:(i + 1) * L] = 1.0
    idn = np.eye(128, dtype=np.float32)
    zro = np.zeros((128, 128), dtype=np.float32)

    in_maps = []
    perms = []
    for c in range(N_CORES):
        own = list(range(IPC * c, IPC * (c + 1)))
        perm = own + [j for j in range(B) if j not in own]
        perms.append(perm)
        ehc = np.ascontiguousarray(ehi[:, perm, :]).reshape(D, B * L)
        elc = np.ascontiguousarray(elo[:, perm, :]).reshape(D, B * L)
        sl = sent[own]                                           # (IPC, L, D)
        sT = np.ascontiguousarray(sl.transpose(2, 0, 1)).reshape(D, IPC * L)
        shc = sT.astype(bf16)
        slc = (sT - shc.astype(np.float32)).astype(bf16)
        w1 = np.linalg.norm(sl, axis=-1).reshape(IPC * L)        # (IPC*L,)
        w1i = np.tile((1.0 / np.maximum(w1, 1e-30)).astype(np.float32), (64, 1))
        in_maps.append(dict(ehi=ehc, elo=elc, shi=shc, slo=slc, w1i=w1i,
                            dlt=dlt, brt=brt, idn=idn, zro=zro))
    return in_maps, perms


def _host_finish(results, perms):
    """Assemble loss and att_maps from per-core outputs."""
    z = np.zeros((B, B), dtype=np.float64)                       # [j, i]
    att = np.zeros((B, 1, L, 1, L), dtype=np.float32)
    for c in range(N_CORES):
        r = results[c]
        perm = perms[c]
        z[perm, IPC * c:IPC * (c + 1)] = r["zout"].astype(np.float64)
        e0s = r["attout"].astype(np.float64)                     # (IPC, s, q)
        a0 = e0s / np.maximum(e0s.sum(axis=2, keepdims=True), 1e-300)
        m = TEMP1 * a0.transpose(0, 2, 1)                        # (IPC, q, s)
        m = m - m.max(axis=2, keepdims=True)
        em = np.exp(m)
        A = em / em.sum(axis=2, keepdims=True)
        att[IPC * c:IPC * (c + 1), 0, :, 0, :] = A.astype(np.float32)

    sim = TEMP3 * np.log(z)                                      # (j, i)

    def nll_diag(s):
        mx = s.max(axis=1, keepdims=True)
        lse = mx[:, 0] + np.log(np.exp(s - mx).sum(axis=1))
        return -(np.diag(s) - lse).mean()

    loss = 0.5 * (nll_diag(sim) + nll_diag(sim.T))
    return np.float32(loss), att


def kernel(ecg_embs, sent_embs):
    ecg = np.asarray(ecg_embs, dtype=np.float32)
    sent = np.asarray(sent_embs, dtype=np.float32)

    if "nc" not in _CACHE:
        _CACHE["nc"] = _build_program()
    nc = _CACHE["nc"]

    in_maps, perms = _host_inputs(ecg, sent)
    trace = bool(int(os.environ.get("BAK_TRACE", "0")))
    res = run_bass_kernel_spmd(nc, in_maps, core_ids=list(range(N_CORES)),
                               trace=trace)
    _CACHE["last_exec_time_ns"] = res.exec_time_ns
    _CACHE["last_results"] = res
    return _host_finish(res.results, perms)


# revision 11
# speedup vs baseline: 1.1185x; 1.1185x over previous
"""Trainium2 Bass kernel for nn_BeatAlignmentModule (cross-attention alignment
loss). Self-contained: takes full inputs, shards over 8 NeuronCores, returns
(loss, att_maps) matching the reference.

Math per (i, j) pair (i = sentence sample, j = ecg sample, all L=64, D=512):
  S[s,q]   = <ecg_j[s,:], sent_i[q,:]>                  (PE, bf16 split-3)
  attn0    = softmax_q(S)       = E0 * rinv0            (ACT exp + DVE)
  X        = exp(T1 * attn0)                            (ACT)
  N[q]     = sum_s X[s,q] * S[s,q]       (numerator <sent, wctx>)
  u[q]     = sum_s X[s,q] * (G_j X)[s,q] (||wctx||^2 via Gram trick)
  cos[q]   = N / (w1 * sqrt(u))   (softmax2 normalizer cancels in cos)
  z[j,i]   = sum_q exp(T2 * cos)
Host: sim = T3*ln(z), symmetric CE loss; att_maps from exported E0 diagonals.

Sharding: data-parallel over i (8 per core); each core sees all j but with a
per-core permutation so its own diagonal j's occupy slots 0..7 (keeps the
device program SPMD-uniform for att-map extraction).
"""
import os
import sys

sys.path.insert(0, "/opt/trn_rl_repo")
os.environ.setdefault("JAX_PLATFORMS", "axon,cpu")

import numpy as np
import ml_dtypes
from contextlib import ExitStack

import concourse.bass as bass
import concourse.tile as tile
from concourse import bacc, mybir
from concourse.bass_utils import run_bass_kernel_spmd

B, L, D = 64, 64, 512
N_CORES = 8
IPC = B // N_CORES            # i's per core
NJP = B // 2                  # j-pairs per core
TEMP1, TEMP2, TEMP3 = 4.0, 5.0, 10.0
CSHIFT = 70.0                 # constant shift inside exp(S - CSHIFT)

F32 = mybir.dt.float32
F32R = mybir.dt.float32r
BF16 = mybir.dt.bfloat16
EXP = mybir.ActivationFunctionType.Exp
LOG = mybir.ActivationFunctionType.Ln
MULT = mybir.AluOpType.mult
ADD = mybir.AluOpType.add
AXX = mybir.AxisListType.X

_CACHE = {}


def _build_program():
    nc = bacc.Bacc("TRN2", target_bir_lowering=False, debug=False,
                   enable_asserts=True, num_devices=N_CORES)

    cb = nc.alloc_sbuf_tensor("const-mshift", [128, 1], F32)
    nc.gpsimd.memset(cb.ap(), -CSHIFT)
    nc.const_aps.aps[(F32, -CSHIFT)] = cb.ap()
    nc.all_engine_barrier()

    # ---- DRAM I/O ----
    ehi_d = nc.dram_tensor("ehi", [D, B * L], BF16, kind="ExternalInput").ap()
    elo_d = nc.dram_tensor("elo", [D, B * L], BF16, kind="ExternalInput").ap()
    shi_d = nc.dram_tensor("shi", [D, IPC * L], BF16, kind="ExternalInput").ap()
    slo_d = nc.dram_tensor("slo", [D, IPC * L], BF16, kind="ExternalInput").ap()
    w1i_d = nc.dram_tensor("w1i", [64, IPC * L], F32, kind="ExternalInput").ap()
    brt_d = nc.dram_tensor("brt", [128, 254], F32R, kind="ExternalInput").ap()
    zro_d = nc.dram_tensor("zro", [128, 128], F32R, kind="ExternalInput").ap()

    z_d = nc.dram_tensor("zout", [64, IPC], F32, kind="ExternalOutput").ap()
    att_d = nc.dram_tensor("attout", [IPC, L, L], F32, kind="ExternalOutput").ap()
    dbg = {}
    if os.environ.get("BAK_DEBUG", "0") == "1":
        for nm, shp in [("d_at", [128, 512]), ("d_x", [128, 512]),
                        ("d_p1", [128, 512]), ("d_p2", [128, 512]),
                        ("d_num", [64, 512]), ("d_uu", [64, 512]),
                        ("d_cos", [64, 512]),
                        ("d_gsb", [128, 128])]:
            dbg[nm] = nc.dram_tensor(nm, shp, F32, kind="ExternalOutput").ap()

    NC, NF = 128, IPC * L  # partition count, wide free dim (512)

    trace_sim = os.environ.get("BAK_TRACESIM", "0") == "1"
    with tile.TileContext(nc, trace_sim=trace_sim) as tc, ExitStack() as ctx:
        cst = ctx.enter_context(tc.tile_pool(name="cst", bufs=1))
        wrk = ctx.enter_context(tc.tile_pool(name="wrk", bufs=2))
        wrk3 = ctx.enter_context(tc.tile_pool(name="wrk3", bufs=3))
        psS = ctx.enter_context(tc.tile_pool(name="psS", bufs=3, space="PSUM"))
        psY = ctx.enter_context(tc.tile_pool(name="psY", bufs=2, space="PSUM"))
        psG = ctx.enter_context(tc.tile_pool(name="psG", bufs=1, space="PSUM"))
        psN = ctx.enter_context(tc.tile_pool(name="psN", bufs=1, space="PSUM"))
        psU = ctx.enter_context(tc.tile_pool(name="psU", bufs=1, space="PSUM"))

        # ---- constants / inputs to SBUF ----
        broute = cst.tile([128, 254], F32R, tag="broute")
        nc.sync.dma_start(broute[:], brt_d[:])
        w1inv = cst.tile([64, NF], F32, tag="w1inv")
        nc.sync.dma_start(w1inv[:], w1i_d[:])

        gsb = [cst.tile([128, 128], F32R, tag=f"gsb{k}", name=f"gsb{k}") for k in range(2)]
        nc.sync.dma_start(gsb[0][:], zro_d[:])
        nc.sync.dma_start(gsb[1][:], zro_d[:])

        shi = []
        slo = []
        for c in range(4):
            th = cst.tile([128, NF], BF16, tag=f"shi{c}")
            nc.sync.dma_start(th[:], shi_d[128 * c:128 * c + 128, :])
            tl = cst.tile([128, NF], BF16, tag=f"slo{c}")
            nc.sync.dma_start(tl[:], slo_d[128 * c:128 * c + 128, :])
            shi.append(th)
            slo.append(tl)

        ehi = [cst.tile([128, B * L], BF16, tag=f"ehi{c}", name=f"ehi{c}") for c in range(4)]
        elo = [cst.tile([128, B * L], BF16, tag=f"elo{c}", name=f"elo{c}") for c in range(4)]
        NSL = 4  # dma column slices
        slw = (B * L) // NSL
        for s in range(NSL):
            for c in range(4):
                nc.sync.dma_start(ehi[c][:, slw * s:slw * (s + 1)],
                                  ehi_d[128 * c:128 * c + 128, slw * s:slw * (s + 1)])
                nc.sync.dma_start(elo[c][:, slw * s:slw * (s + 1)],
                                  elo_d[128 * c:128 * c + 128, slw * s:slw * (s + 1)])

        # ---- main loop over j-pairs ----
        reps = int(os.environ.get("BAK_REPS", "1"))
        rep_cm = tc.For_i(0, reps, 1) if reps > 1 else None
        if rep_cm is not None:
            rep_cm.__enter__()
        pend = None  # deferred (x, p1, p2s, jj) for Y/colsum of previous jj
        nnum = psN.tile([128, NF], F32, tag="nnum")
        nuu = psU.tile([128, NF], F32, tag="nuu")

        def emit_deferred(p):
            x, p1t, jjp = p
            # Y = blockdiag(G) @ X
            py = psY.tile([128, NF], F32, tag="psy")
            nc.tensor.matmul(py[:], gsb[jjp % 2][:], x[:])
            # P2 = X * Y
            p2t = wrk.tile([128, NF], F32R, tag="p2")
            nc.vector.tensor_tensor(p2t[:], x[:], py[:], op=MULT)
            if dbg and jjp == 0:
                nc.sync.dma_start(dbg["d_p2"][:], p2t[:].bitcast(F32))
                nc.sync.dma_start(dbg["d_gsb"][:], gsb[jjp % 2][:].bitcast(F32))
            # colsum accumulations
            o = 126 - 2 * jjp
            nc.tensor.matmul(nnum[:], broute[:, o:o + 128], p1t[:],
                             start=(jjp == 0), stop=(jjp == NJP - 1),
                             skip_group_check=True)
            nc.tensor.matmul(nuu[:], broute[:, o:o + 128], p2t[:],
                             start=(jjp == 0), stop=(jjp == NJP - 1),
                             skip_group_check=True)

        pend2 = None
        for jj in range(NJP):
            jc = slice(128 * jj, 128 * jj + 128)
            # --- Gram of the ecg j-pair (bf16 hi only) ---
            pg = psG.tile([128, 128], F32, tag="pg")
            for c in range(4):
                nc.tensor.matmul(pg[:], ehi[c][:, jc], ehi[c][:, jc],
                                 start=(c == 0), stop=(c == 3))
            # copy diagonal blocks into the (pre-zeroed) block-diag tile
            nc.scalar.copy(gsb[jj % 2][0:64, 0:64], pg[0:64, 0:64])
            nc.scalar.copy(gsb[jj % 2][64:128, 64:128], pg[64:128, 64:128])

            # --- MM1: S = ecgT^T @ sentT, bf16 split-3 ---
            ps = psS.tile([128, NF], F32, tag="ps")
            k = 0
            for lh, rh in ((ehi, shi), (ehi, slo), (elo, shi)):
                for c in range(4):
                    nc.tensor.matmul(ps[:], lh[c][:, jc], rh[c][:],
                                     start=(k == 0), stop=(k == 11))
                    k += 1

            # --- deferred Y / colsums, 2 j-pairs back (keeps PE dense) ---
            if pend2 is not None:
                emit_deferred(pend2)
            pend2 = pend

            # --- softmax1 pieces ---
            e0r = wrk.tile([128, NF], F32, tag="e0r")
            nc.scalar.activation(e0r[:], ps[:], EXP, bias=-CSHIFT, scale=1.0)
            # clamp: exp can overflow to inf for |S| ~ 180; softmax saturates
            # so clamping is harmless (err ~1e-9). Runs on idle GPSIMD.
            e0 = wrk.tile([128, NF], F32, tag="e0")
            nc.gpsimd.tensor_scalar_min(e0[:], e0r[:], 1e35)
            r0 = wrk.tile([128, IPC], F32, tag="r0")
            nc.vector.tensor_reduce(r0[:], e0[:].rearrange("p (a b) -> p a b", a=IPC),
                                    axis=AXX, op=ADD)
            rinv = wrk.tile([128, IPC], F32, tag="rinv")
            nc.vector.reciprocal(rinv[:], r0[:])

            # att-map source export (diagonal pairs live in slots 0..7)
            if jj < IPC // 2:
                for h in (0, 1):
                    kb = 2 * jj + h
                    nc.sync.dma_start(att_d[kb],
                                      e0[64 * h:64 * h + 64, 64 * kb:64 * kb + 64])

            # --- attn0 = E0 * rinv broadcast along q (stride-0 DVE view) ---
            at = wrk.tile([128, NF], F32, tag="at")
            rview = rinv[:].rearrange("p (a b) -> p a b", a=IPC)                            .to_broadcast([128, IPC, 64])
            nc.vector.tensor_tensor(at[:].rearrange("p (a b) -> p a b", a=IPC),
                                    e0[:].rearrange("p (a b) -> p a b", a=IPC),
                                    rview, op=MULT)
            if dbg and jj == 0:
                nc.sync.dma_start(dbg["d_at"][:], at[:])

            # --- X = exp(T1 * attn0) ---
            x = wrk3.tile([128, NF], F32R, tag="x")
            nc.scalar.activation(x[:], at[:], EXP, bias=0.0, scale=TEMP1)

            # --- P1 = X * S ---
            p1t = wrk3.tile([128, NF], F32R, tag="p1")
            nc.vector.tensor_tensor(p1t[:], x[:], ps[:], op=MULT)
            if dbg and jj == 0:
                nc.sync.dma_start(dbg["d_x"][:], x[:].bitcast(F32))
                nc.sync.dma_start(dbg["d_p1"][:], p1t[:].bitcast(F32))

            pend = (x, p1t, jj)

        emit_deferred(pend2)
        emit_deferred(pend)

        # ---- tail: cos, z ----
        lu = wrk.tile([64, NF], F32, tag="lu")
        nc.scalar.activation(lu[:], nuu[0:64, :], LOG)
        isq = wrk.tile([64, NF], F32, tag="isq")
        nc.scalar.activation(isq[:], lu[:], EXP, bias=0.0, scale=-0.5)
        c1 = wrk.tile([64, NF], F32, tag="c1")
        nc.vector.tensor_tensor(c1[:], nnum[0:64, :], w1inv[:], op=MULT)
        cosv = wrk.tile([64, NF], F32, tag="cosv")
        nc.vector.tensor_tensor(cosv[:], c1[:], isq[:], op=MULT)
        if dbg:
            dnum = wrk.tile([64, NF], F32, tag="dnum")
            nc.vector.tensor_copy(dnum[:], nnum[0:64, :])
            nc.sync.dma_start(dbg["d_num"][:], dnum[:])
            duu = wrk.tile([64, NF], F32, tag="duu")
            nc.vector.tensor_copy(duu[:], nuu[0:64, :])
            nc.sync.dma_start(dbg["d_uu"][:], duu[:])
            nc.sync.dma_start(dbg["d_cos"][:], cosv[:])
        ez = wrk.tile([64, NF], F32, tag="ez")
        nc.scalar.activation(ez[:], cosv[:], EXP, bias=0.0, scale=TEMP2)
        zt = wrk.tile([64, IPC], F32, tag="zt")
        nc.vector.tensor_reduce(zt[:], ez[:].rearrange("p (a b) -> p a b", a=IPC),
                                axis=AXX, op=ADD)
        nc.sync.dma_start(z_d[:], zt[:])
        if rep_cm is not None:
            rep_cm.__exit__(None, None, None)

    nc.compile()
    return nc


def _host_inputs(ecg, sent):
    """Build per-core input maps. ecg/sent: (B, L, D) float32."""
    bf16 = ml_dtypes.bfloat16
    ecgT = np.ascontiguousarray(ecg.transpose(2, 0, 1))          # (D, B, L)
    ehi = ecgT.astype(bf16)
    elo = (ecgT - ehi.astype(np.float32)).astype(bf16)

    brt = np.zeros((128, 254), dtype=np.float32)
    brt[0:64, 126] = 1.0
    brt[64:128, 127] = 1.0
    zro = np.zeros((128, 128), dtype=np.float32)

    in_maps = []
    perms = []
    for c in range(N_CORES):
        own = list(range(IPC * c, IPC * (c + 1)))
        perm = own + [j for j in range(B) if j not in own]
        perms.append(perm)
        ehc = np.ascontiguousarray(ehi[:, perm, :]).reshape(D, B * L)
        elc = np.ascontiguousarray(elo[:, perm, :]).reshape(D, B * L)
        sl = sent[own]                                           # (IPC, L, D)
        sT = np.ascontiguousarray(sl.transpose(2, 0, 1)).reshape(D, IPC * L)
        shc = sT.astype(bf16)
        slc = (sT - shc.astype(np.float32)).astype(bf16)
        w1 = np.linalg.norm(sl, axis=-1).reshape(IPC * L)        # (IPC*L,)
        w1i = np.tile((1.0 / np.maximum(w1, 1e-30)).astype(np.float32), (64, 1))
        in_maps.append(dict(ehi=ehc, elo=elc, shi=shc, slo=slc, w1i=w1i,
                            brt=brt, zro=zro))
    return in_maps, perms


def _host_finish(results, perms):
    """Assemble loss and att_maps from per-core outputs."""
    z = np.zeros((B, B), dtype=np.float64)                       # [j, i]
    att = np.zeros((B, 1, L, 1, L), dtype=np.float32)
    for c in range(N_CORES):
        r = results[c]
        perm = perms[c]
        z[perm, IPC * c:IPC * (c + 1)] = r["zout"].astype(np.float64)
        e0s = r["attout"].astype(np.float64)                     # (IPC, s, q)
        a0 = e0s / np.maximum(e0s.sum(axis=2, keepdims=True), 1e-300)
        m = TEMP1 * a0.transpose(0, 2, 1)                        # (IPC, q, s)
        m = m - m.max(axis=2, keepdims=True)
        em = np.exp(m)
        A = em / em.sum(axis=2, keepdims=True)
        att[IPC * c:IPC * (c + 1), 0, :, 0, :] = A.astype(np.float32)

    sim = TEMP3 * np.log(z)                                      # (j, i)

    def nll_diag(s):
        mx = s.max(axis=1, keepdims=True)
        lse = mx[:, 0] + np.log(np.exp(s - mx).sum(axis=1))
        return -(np.diag(s) - lse).mean()

    loss = 0.5 * (nll_diag(sim) + nll_diag(sim.T))
    return np.float32(loss), att


def kernel(ecg_embs, sent_embs):
    ecg = np.asarray(ecg_embs, dtype=np.float32)
    sent = np.asarray(sent_embs, dtype=np.float32)

    if "nc" not in _CACHE:
        _CACHE["nc"] = _build_program()
    nc = _CACHE["nc"]

    in_maps, perms = _host_inputs(ecg, sent)
    trace = bool(int(os.environ.get("BAK_TRACE", "0")))
    res = run_bass_kernel_spmd(nc, in_maps, core_ids=list(range(N_CORES)),
                               trace=trace)
    _CACHE["last_exec_time_ns"] = res.exec_time_ns
    _CACHE["last_results"] = res
    return _host_finish(res.results, perms)
